# revision 25
# baseline (speedup 1.0000x reference)
"""Additive (Bahdanau) attention on 8 TRN2 NeuronCores — self-contained Bass kernel.

Math: score(q,k) = w2 . tanh(hq[q] + hk[k] + b1) + b2;  out = softmax_k(score) @ V.

tanh(s) ~= sum_m c_m sin(w_m s) with a DOUBLING basis w_m = {1,2,4,8}*w0
(weighted-LSQ fit; e2e rel-err ~4.6e-3 in full bf16 simulation).  Angle
addition sin(w(a+b)) = sin(wa)cos(wb)+cos(wa)sin(wb) turns the [B,Q,K,D]
tanh+reduce into TensorE matmuls contracting over (2M x D).

Per side (F=queries, G=keys), features at level scales [alpha]:
  m0: s1=sin(w0 h) [1],   c1=sin(w0 h + pi/2) [1]     (ScalarE, from PSUM)
  m1: S2=s1*c1 [1/2],     C2=0.5-s1^2 [1/2]           (DVE bf16 products)
  m2: S4=S2*C2 [1/8],     C4=0.125-S2^2 [1/8]
  m3: S8=S4*C4 [1/128],   C8=1/128-S4^2 [1/128]
b1 folds into the hk matmul via a rank-1 ones-outer-product row;  w2 and c_m
fold into per-partition scales of the F-side features (sF tiles); the m3 sin
scaled feature is fused via scalar_tensor_tensor (S8F never materializes).
All Sin args within [-pi,pi]: no range reduction (CoreSim-safe).

Softmax: b2 drops (shift invariance); the denominator comes FREE from attn@V
by appending a ones-column to V ([P,257] matmul), reciprocal on VectorE,
folded into a per-q output scale.

Structure: inputs split over 3 DMA queues; dummy Sin hoists the trig table
load to t~0; ~2.5us of warm-up matmuls keep the PE's HAM clock at 2.4GHz;
Sin reads h straight from PSUM; per-(side,b) transposes start on the first
DMA chunk; Pool (GpSimd) absorbs p2 squares and the m0 feature scaling.

Sharding: data-parallel over batch, B=16 -> 2 per core, no collectives.
"""

import math
from contextlib import ExitStack

import numpy as np
import ml_dtypes

import concourse.bass as bass
import concourse.mybir as mybir
import concourse.tile as tile
from concourse import bacc
from concourse.bass_utils import run_bass_kernel_spmd

F32 = mybir.dt.float32
BF16 = mybir.dt.bfloat16
AF = mybir.ActivationFunctionType
ALU = mybir.AluOpType

NCORES = 8
B, NQ, NK, D = 16, 256, 256, 256
BL = B // NCORES          # local batches per core = 2
P = 128
DC = D // P               # d-chunks = 2
EC = D // P               # e-chunks (contraction for hq/hk matmuls) = 2
QT = NQ // P              # q-tiles = 2
KT = NK // P              # k-tiles = 2
M_SINES = 4
W = BL * NQ               # 512: free width per dt slice
WF = DC * W               # 1024: per-side width (F half [0,WF), G half [WF,2WF))
DV1 = D + 1               # values + ones column
NWARM = 40                # PE warm-up matmuls

# {1,2,4,8}*W0 weighted-LSQ fit of tanh (Gaussian(~1.0)+5e-4 weight)
W0 = 0.378
COEF = (1.186435, 0.13547, 0.228208, 0.032448)
ALPHA2 = (1.0, 0.25, 1.0 / 64, 1.0 / 16384)   # alpha_sin*alpha_cos per m

# tbl columns: pi/2, G-side Sin biases, then per-m F-side multipliers per dt
TB_HPI = 0
TB_WB1 = 1                 # [dt] w0*b1
TB_WB1H = 1 + DC           # [dt] w0*b1 + pi/2
TB_W2C = 1 + 2 * DC        # [m*DC+dt] c_m*w2/alpha2_m
TB_C4B = TB_W2C + M_SINES * DC   # 0.125
TB_C8B = TB_C4B + 1              # 1/128
TB_N = TB_C8B + 1


def build_kernel() -> bacc.Bacc:
    nc = bacc.Bacc("TRN2", target_bir_lowering=False, debug=False)

    q_d = nc.dram_tensor("queries", [BL, NQ, D], BF16, kind="ExternalInput").ap()
    k_d = nc.dram_tensor("keys", [BL, NK, D], BF16, kind="ExternalInput").ap()
    v_d = nc.dram_tensor("values", [BL, NK, D], BF16, kind="ExternalInput").ap()
    wqk_d = nc.dram_tensor("wqk", [P, 2 * EC * D], BF16, kind="ExternalInput").ap()
    tbl_d = nc.dram_tensor("tbl", [P, TB_N], F32, kind="ExternalInput").ap()
    id_d = nc.dram_tensor("ident", [P, P], BF16, kind="ExternalInput").ap()
    out_d = nc.dram_tensor("out", [BL, NQ, D], F32, kind="ExternalOutput").ap()

    with tile.TileContext(nc) as tc, ExitStack() as ctx:
        cpool = ctx.enter_context(tc.tile_pool(name="consts", bufs=1))
        dpool = ctx.enter_context(tc.tile_pool(name="data", bufs=1))

        # dummy 1-col Sin hoists the trig ACT-table load to program start;
        # the same tile feeds the PE warm-up matmuls
        dummy = cpool.tile([P, 132], BF16)
        nc.vector.memset(dummy[:], 0.0)
        nc.scalar.activation(dummy[:, 130:131], dummy[:, 129:130], AF.Sin)

        ident = cpool.tile([P, P], BF16)
        wqk = cpool.tile([P, 2 * EC * D], BF16)
        tbl = cpool.tile([P, TB_N], F32)
        qnb = dpool.tile([P, BL * QT * D], BF16)
        knb = dpool.tile([P, BL * KT * D], BF16)
        vb = dpool.tile([P, BL * KT * DV1], BF16)

        # ---- input DMAs: 3 queues (sync/scalar/gpsimd) in parallel ----
        nc.sync.dma_start(
            qnb[:, 0:QT * D].rearrange("p (t e) -> p t e", t=QT),
            q_d[0].rearrange("(t p) e -> p t e", p=P))
        nc.scalar.dma_start(ident[:], id_d[:])
        nc.scalar.dma_start(
            qnb[:, QT * D:2 * QT * D].rearrange("p (t e) -> p t e", t=QT),
            q_d[1].rearrange("(t p) e -> p t e", p=P))
        nc.scalar.dma_start(wqk[:], wqk_d[:])
        nc.gpsimd.dma_start(
            knb[:, 0:KT * D].rearrange("p (t e) -> p t e", t=KT),
            k_d[0].rearrange("(t p) e -> p t e", p=P))
        nc.gpsimd.dma_start(tbl[:], tbl_d[:])
        nc.sync.dma_start(
            knb[:, KT * D:2 * KT * D].rearrange("p (t e) -> p t e", t=KT),
            k_d[1].rearrange("(t p) e -> p t e", p=P))
        nc.gpsimd.dma_start(
            vb[:].rearrange("p (b t e) -> p b t e", b=BL, t=KT)[:, :, :, 0:D],
            v_d.rearrange("b (t p) e -> p b t e", p=P))
        nc.gpsimd.memset(
            vb[:].rearrange("p (b t e) -> p b t e", b=BL, t=KT)[:, :, :, D:DV1], 1.0)

        halfpi = tbl[:, TB_HPI:TB_HPI + 1]

        def w2c(m, dt):
            col = TB_W2C + m * DC + dt
            return tbl[:, col:col + 1]

        # transposed inputs (bf16): col = (ec*BL + b)*256 + q
        qTt = dpool.tile([P, EC * BL * NQ], BF16)
        kTt = dpool.tile([P, EC * BL * NK], BF16)

        # trig feature tiles: F half [0,WF), G half [WF,2WF);
        # within a half: col = dt*W + b*NQ + q
        s1 = dpool.tile([P, 2 * WF], BF16)
        c1 = dpool.tile([P, 2 * WF], BF16)
        S2 = dpool.tile([P, 2 * WF], BF16)
        C2 = dpool.tile([P, 2 * WF], BF16)
        S4 = dpool.tile([P, 2 * WF], BF16)
        C4 = dpool.tile([P, 2 * WF], BF16)
        S8 = dpool.tile([P, WF], BF16)          # G half only (F side fused)
        C8 = dpool.tile([P, 2 * WF], BF16)
        ppool = ctx.enter_context(tc.tile_pool(name="prods", bufs=3))
        sfpool = ctx.enter_context(tc.tile_pool(name="scaledF", bufs=2))

        with tc.tile_pool(name="warm", bufs=1, space="PSUM") as wmpool:
            wm = wmpool.tile([P, 1], F32, name="wm", tag="wm")
            for _ in range(NWARM):
                nc.tensor.matmul(wm[:], dummy[:, 0:P], dummy[:, 128:129],
                                 start=True, stop=True)

        with tc.tile_pool(name="tpsum", bufs=4, space="PSUM") as tpool:
            # transposes per (side, b, j): b=1's q chunk (scalar queue) lands first
            for (side, natb, dst, border) in (
                    (0, qnb, qTt, (1, 0)), (1, knb, kTt, (0, 1))):
                for b in border:
                    for j in range(EC):
                        tp = tpool.tile([P, QT * P], BF16, name=f"tp{side}{b}{j}", tag="tp")
                        for i in range(QT):
                            nc.tensor.transpose(
                                tp[:, i * P:(i + 1) * P],
                                natb[:, (b * QT + i) * D + j * P:(b * QT + i) * D + (j + 1) * P],
                                ident)
                        nc.vector.tensor_copy(
                            dst[:, (j * BL + b) * NQ:(j * BL + b + 1) * NQ], tp[:])

        with tc.tile_pool(name="hpsum", bufs=2, space="PSUM") as hpool:
            # hq then hk matmuls into PSUM; Sin reads PSUM directly
            h_f = hpool.tile([P, 2 * W], F32, name="h_f", tag="h")
            h_g = hpool.tile([P, 2 * W], F32, name="h_g", tag="h")
            for dt in range(DC):
                for b in range(BL):
                    for ec in range(EC):
                        nc.tensor.matmul(
                            h_f[:, dt * W + b * NQ:dt * W + (b + 1) * NQ],
                            wqk[:, ec * D + dt * P:ec * D + (dt + 1) * P],
                            qTt[:, (ec * BL + b) * NQ:(ec * BL + b + 1) * NQ],
                            start=(ec == 0), stop=(ec == EC - 1))
            # F-side base trig (PSUM-source) as soon as h_f is done
            nc.scalar.activation(s1[:, 0:WF], h_f[:], AF.Sin, bias=0.0, scale=W0)
            nc.scalar.activation(c1[:, 0:WF], h_f[:], AF.Sin, bias=halfpi, scale=W0)

            for dt in range(DC):
                for b in range(BL):
                    for ec in range(EC):
                        nc.tensor.matmul(
                            h_g[:, dt * W + b * NQ:dt * W + (b + 1) * NQ],
                            wqk[:, EC * D + ec * D + dt * P:EC * D + ec * D + (dt + 1) * P],
                            kTt[:, (ec * BL + b) * NQ:(ec * BL + b + 1) * NQ],
                            start=(ec == 0), stop=(ec == EC - 1))
            for dt in range(DC):
                nc.scalar.activation(
                    s1[:, WF + dt * W:WF + (dt + 1) * W], h_g[:, dt * W:(dt + 1) * W],
                    AF.Sin, bias=tbl[:, TB_WB1 + dt:TB_WB1 + dt + 1], scale=W0)
                nc.scalar.activation(
                    c1[:, WF + dt * W:WF + (dt + 1) * W], h_g[:, dt * W:(dt + 1) * W],
                    AF.Sin, bias=tbl[:, TB_WB1H + dt:TB_WB1H + dt + 1], scale=W0)

        wpool = ctx.enter_context(tc.tile_pool(name="wpsum", bufs=2, space="PSUM"))
        avpool = ctx.enter_context(tc.tile_pool(name="avpsum", bufs=1, space="PSUM"))
        logits_ps = [wpool.tile([P, BL * NQ], F32, name=f"lg_{kt}", tag="lg")
                     for kt in range(KT)]
        expT = dpool.tile([P, KT * BL * NQ], BF16)
        sFs = [sfpool.tile([P, 2 * WF], BF16, name=f"sF{m}", tag=f"sF{m % 2}")
               for m in range(M_SINES)]

        FH = slice(0, WF)
        GH = slice(WF, 2 * WF)

        # ---- m0 F-feature scaling (DVE, ready earliest) ----
        for dt in range(DC):
            nc.vector.tensor_scalar_mul(
                sFs[0][:, dt * W:(dt + 1) * W], s1[:, dt * W:(dt + 1) * W], w2c(0, dt))
            nc.vector.tensor_scalar_mul(
                sFs[0][:, WF + dt * W:WF + (dt + 1) * W],
                c1[:, dt * W:(dt + 1) * W], w2c(0, dt))

        # ---- F chain start (DVE) ----
        p1f = ppool.tile([P, WF], BF16, name="p1f", tag="pp")
        nc.vector.tensor_tensor(p1f[:], s1[:, FH], s1[:, FH], op=ALU.mult)
        nc.vector.tensor_tensor(S2[:, FH], s1[:, FH], c1[:, FH], op=ALU.mult)
        nc.vector.tensor_scalar(C2[:, FH], p1f[:], -1.0, 0.5, op0=ALU.mult, op1=ALU.add)
        for dt in range(DC):                       # sF1
            nc.vector.tensor_scalar_mul(
                sFs[1][:, dt * W:(dt + 1) * W], S2[:, dt * W:(dt + 1) * W], w2c(1, dt))
            nc.vector.tensor_scalar_mul(
                sFs[1][:, WF + dt * W:WF + (dt + 1) * W],
                C2[:, dt * W:(dt + 1) * W], w2c(1, dt))
        nc.vector.tensor_tensor(S4[:, FH], S2[:, FH], C2[:, FH], op=ALU.mult)

        # ---- G chain start (DVE) so ScalarE's p2g sees S2[:,GH] written ----
        p1g = ppool.tile([P, WF], BF16, name="p1g", tag="pp")
        nc.vector.tensor_tensor(p1g[:], s1[:, GH], s1[:, GH], op=ALU.mult)
        nc.vector.tensor_tensor(S2[:, GH], s1[:, GH], c1[:, GH], op=ALU.mult)
        nc.vector.tensor_scalar(C2[:, GH], p1g[:], -1.0, 0.5, op0=ALU.mult, op1=ALU.add)

        # ---- helpers on ScalarE (Square/Identity live in the exp set) ----
        p2g = ppool.tile([P, WF], BF16, name="p2g", tag="pp")
        nc.scalar.activation(p2g[:], S2[:, GH], AF.Square)
        p2f = ppool.tile([P, WF], BF16, name="p2f", tag="pp")
        nc.scalar.activation(p2f[:], S2[:, FH], AF.Square)
        nc.scalar.activation(C4[:, FH], p2f[:], AF.Identity,
                             bias=tbl[:, TB_C4B:TB_C4B + 1], scale=-1.0)
        p3f = ppool.tile([P, WF], BF16, name="p3f", tag="pp")
        nc.scalar.activation(p3f[:], S4[:, FH], AF.Square)
        nc.scalar.activation(dummy[:, 131:132], p3f[:, WF - 1:WF], AF.Exp)
        nc.vector.tensor_scalar(C8[:, FH], p3f[:], -1.0, 1.0 / 128,
                                op0=ALU.mult, op1=ALU.add)

        # ---- G chain rest (DVE) ----
        nc.vector.tensor_tensor(S4[:, GH], S2[:, GH], C2[:, GH], op=ALU.mult)
        p3g = ppool.tile([P, WF], BF16, name="p3g", tag="pp")
        nc.vector.tensor_tensor(p3g[:], S4[:, GH], S4[:, GH], op=ALU.mult)
        nc.vector.tensor_scalar(C8[:, GH], p3g[:], -1.0, 1.0 / 128, op0=ALU.mult, op1=ALU.add)
        for dt in range(DC):                       # sF2 (C4F from ScalarE)
            nc.vector.tensor_scalar_mul(
                sFs[2][:, dt * W:(dt + 1) * W], S4[:, dt * W:(dt + 1) * W], w2c(2, dt))
            nc.vector.tensor_scalar_mul(
                sFs[2][:, WF + dt * W:WF + (dt + 1) * W],
                C4[:, dt * W:(dt + 1) * W], w2c(2, dt))
        nc.vector.tensor_scalar(C4[:, GH], p2g[:], -1.0, 0.125, op0=ALU.mult, op1=ALU.add)
        nc.vector.tensor_tensor(S8[:], S4[:, GH], C4[:, GH], op=ALU.mult)
        for dt in range(DC):                       # sF3: sin fused via STT
            nc.vector.scalar_tensor_tensor(
                sFs[3][:, dt * W:(dt + 1) * W],
                S4[:, dt * W:(dt + 1) * W], w2c(3, dt), C4[:, dt * W:(dt + 1) * W],
                op0=ALU.mult, op1=ALU.mult)
            nc.vector.tensor_scalar_mul(
                sFs[3][:, WF + dt * W:WF + (dt + 1) * W],
                C8[:, dt * W:(dt + 1) * W], w2c(3, dt))

        # ---- logits matmuls ----
        def lg_mm(m, kt, pi_, dt, b, gtile, goff, start, stop):
            nc.tensor.matmul(
                logits_ps[kt][:, b * NQ:(b + 1) * NQ],
                gtile[:, goff + dt * W + b * NQ + kt * P:goff + dt * W + b * NQ + (kt + 1) * P],
                sFs[m][:, pi_ * WF + dt * W + b * NQ:pi_ * WF + dt * W + (b + 1) * NQ],
                start=start, stop=stop)

        def logits_m(m, gsin, soff, gcos, coff, last):
            if not last:
                for pi_ in range(2):
                    gtile, goff = (gcos, coff) if pi_ == 0 else (gsin, soff)
                    for dt in range(DC):
                        for b in range(BL):
                            for kt in range(KT):
                                lg_mm(m, kt, pi_, dt, b, gtile, goff,
                                      m == 0 and pi_ == 0 and dt == 0, False)
            else:
                # pi=1 (cos-F x sin-G) first: its operands land before the
                # fused sF3-sin STT that pi=0 needs
                for kt in range(KT):
                    for pi_ in (1, 0):
                        gtile, goff = (gcos, coff) if pi_ == 0 else (gsin, soff)
                        for dt in range(DC):
                            for b in range(BL):
                                lg_mm(m, kt, pi_, dt, b, gtile, goff, False,
                                      pi_ == 0 and dt == DC - 1)
                    nc.scalar.activation(
                        expT[:, kt * BL * NQ:(kt + 1) * BL * NQ],
                        logits_ps[kt][:], AF.Exp)

        logits_m(0, s1, WF, c1, WF, False)
        logits_m(1, S2, WF, C2, WF, False)
        logits_m(2, S4, WF, C4, WF, False)
        logits_m(3, S8, 0, C8, WF, True)

        # ---- attn @ [V|1]: denominator rides in column 256 ----
        out_sb = dpool.tile([P, BL * QT * D], F32)
        rcol = cpool.tile([P, BL * QT], F32)
        avt = avpool.tile([P, BL * QT * 512], F32, name="av", tag="av")
        for kt in range(KT):
            for b in range(BL):
                for qt in range(QT):
                    r = (b * QT + qt) * 512
                    nc.tensor.matmul(
                        avt[:, r:r + DV1],
                        expT[:, (kt * BL + b) * NQ + qt * P:(kt * BL + b) * NQ + (qt + 1) * P],
                        vb[:, (b * KT + kt) * DV1:(b * KT + kt + 1) * DV1],
                        start=(kt == 0), stop=(kt == KT - 1))
        # two batched reciprocals over denominator columns, in completion order
        nc.vector.reciprocal(
            rcol[:, 0:2],
            avt[:, 0:1024].rearrange("p (r c) -> p r c", r=2)[:, :, D:D + 1])
        nc.vector.reciprocal(
            rcol[:, 2:4],
            avt[:, 1024:2048].rearrange("p (r c) -> p r c", r=2)[:, :, D:D + 1])
        for b in range(BL):
            for qt in range(QT):
                i = b * QT + qt
                av = avt[:, i * 512:i * 512 + DV1]
                rc = rcol[:, i:i + 1]
                osl = out_sb[:, i * D:(i + 1) * D]
                if i % 2 == 0:
                    nc.scalar.activation(osl, av[:, 0:D], AF.Copy, bias=0.0, scale=rc)
                else:
                    nc.vector.tensor_scalar_mul(osl, av[:, 0:D], rc)
                eng = (nc.sync, nc.gpsimd, nc.scalar, nc.gpsimd)[i]
                eng.dma_start(out_d[b, qt * P:(qt + 1) * P, :], osl)

    nc.compile()
    return nc


def _host_tables(b1: np.ndarray, w2: np.ndarray):
    tbl = np.zeros((P, TB_N), np.float32)
    tbl[:, TB_HPI] = math.pi / 2.0
    tbl[:, TB_C4B] = 0.125
    tbl[:, TB_C8B] = 1.0 / 128.0
    for dt in range(DC):
        tbl[:, TB_WB1 + dt] = W0 * b1[dt * P:(dt + 1) * P]
        tbl[:, TB_WB1H + dt] = W0 * b1[dt * P:(dt + 1) * P] + math.pi / 2.0
        for mi in range(M_SINES):
            tbl[:, TB_W2C + mi * DC + dt] = COEF[mi] * w2[dt * P:(dt + 1) * P] / ALPHA2[mi]
    return tbl


_NC_CACHE = {}


def _get_nc():
    if "nc" not in _NC_CACHE:
        _NC_CACHE["nc"] = build_kernel()
    return _NC_CACHE["nc"]


def _make_in_maps(inputs):
    keys = np.ascontiguousarray(np.asarray(inputs["keys"], np.float32).astype(ml_dtypes.bfloat16))
    queries = np.ascontiguousarray(np.asarray(inputs["queries"], np.float32).astype(ml_dtypes.bfloat16))
    values = np.ascontiguousarray(np.asarray(inputs["values"], np.float32).astype(ml_dtypes.bfloat16))
    Wk = np.asarray(inputs["Wk"], np.float32)
    Wq = np.asarray(inputs["Wq"], np.float32)
    b1 = np.asarray(inputs["b1"], np.float64)
    w2 = np.asarray(inputs["w2"], np.float64)

    wqk = np.concatenate(
        [Wq.reshape(EC, P, D).transpose(1, 0, 2).reshape(P, EC * D),
         Wk.reshape(EC, P, D).transpose(1, 0, 2).reshape(P, EC * D)],
        axis=1).astype(ml_dtypes.bfloat16)
    wqk = np.ascontiguousarray(wqk)
    tbl = _host_tables(b1, w2)
    ident = np.ascontiguousarray(np.eye(P, dtype=np.float32).astype(ml_dtypes.bfloat16))

    in_maps = []
    for c in range(NCORES):
        sl = slice(c * BL, (c + 1) * BL)
        in_maps.append({
            "queries": queries[sl], "keys": keys[sl], "values": values[sl],
            "wqk": wqk, "tbl": tbl, "ident": ident,
        })
    return in_maps


def _run(inputs, trace=False, trace_kwargs=None):
    nc = _get_nc()
    in_maps = _make_in_maps(inputs)
    kwargs = {}
    if trace:
        kwargs = dict(trace=True, trace_cores=[0], trace_kwargs=trace_kwargs or {})
    res = run_bass_kernel_spmd(nc, in_maps, core_ids=list(range(NCORES)), **kwargs)
    out = np.concatenate([res.results[c]["out"] for c in range(NCORES)], axis=0)
    return out, res


def kernel(**inputs) -> np.ndarray:
    out, _ = _run(inputs, trace=False)
    return out
